# revision 21
# baseline (speedup 1.0000x reference)
"""Multi-head attention (16 heads, head-parallel over 8 NeuronCores) in Bass/Tile.

Sharding: 2 heads per core. Each core computes its heads' QKV projections,
raw attention scores (an output), softmax, attention-weighted values, and a
row-sharded slice of the final linear; partial products are summed with an
on-device AllReduce. Host only packs per-core weights, transposes x once,
and concatenates per-core att outputs.

Layout notes (T suffix = feature-major, i.e. transposed vs torch layout):
 - xT        [D, M]        M = B*S; moving operand for projections
 - QT/KT/VT  [128, S]      per b; rows 0:64 head0, 64:128 head1
 - att[q,s]  via matmul(lhsT=QT block, rhs=KT)     -> raw scores output
 - attT[s,q] via matmul(lhsT=KT block, rhs=QT)     -> exp() -> PT
 - o^T       via matmul(lhsT=V_aug[s, 65], rhs=PT) ; col 64 of V_aug is 1.0
              so o^T row 64 = softmax denominator (unnormalized row sums)
 - all matmul operands are float32r (FP22 mantissa) for 4x PE throughput;
   accumulation stays fp32 in PSUM.
"""

import sys

sys.path.insert(0, "/opt/trn_rl_repo")

from contextlib import ExitStack

import numpy as np

import concourse.bass as bass  # noqa: F401  (AP types)
import concourse.tile as tile
from concourse import bacc, mybir
from concourse.bass_utils import run_bass_kernel_spmd

F32 = mybir.dt.float32
F32R = mybir.dt.float32r
AF = mybir.ActivationFunctionType

N_CORES = 8


def build_program(B, S, D, dk=64, hpc=2):
    M = B * S
    DC = D // 128                      # contraction chunks for projections
    TQ = 512 if S % 512 == 0 else 256  # moving-operand tile size
    NT = S // TQ                       # moving tiles per sequence
    QB = S // 128                      # 128-row blocks per sequence
    HF = hpc * dk                      # features per core (128)
    assert HF == 128 and D % 128 == 0 and S % 256 == 0
    inv_sqrt_dk = 1.0 / float(np.sqrt(dk))

    nc = bacc.Bacc("TRN2", target_bir_lowering=False, debug=False,
                   num_devices=N_CORES)

    TQ0 = 512 if S % 512 == 0 else 256
    xT = nc.dram_tensor("xT", [128, B, S // TQ0, D // 128, TQ0], F32R,
                        kind="ExternalInput").ap()
    wq = nc.dram_tensor("wq", [D, HF], F32R, kind="ExternalInput").ap()
    wk = nc.dram_tensor("wk", [D, HF], F32R, kind="ExternalInput").ap()
    wv = nc.dram_tensor("wv", [D, HF], F32R, kind="ExternalInput").ap()
    bq = nc.dram_tensor("bq", [HF, 1], F32, kind="ExternalInput").ap()
    bk = nc.dram_tensor("bk", [HF, 1], F32, kind="ExternalInput").ap()
    bv = nc.dram_tensor("bv", [HF, 1], F32, kind="ExternalInput").ap()
    wo = nc.dram_tensor("wo", [HF, dk], F32R, kind="ExternalInput").ap()
    bo = nc.dram_tensor("bo", [dk // N_CORES, 1], F32,
                        kind="ExternalInput").ap()
    ident = nc.dram_tensor("ident", [128, 128], F32R, kind="ExternalInput").ap()
    att_out = nc.dram_tensor("att_out", [B, hpc, S, S], F32,
                             kind="ExternalOutput").ap()
    dkr = dk // N_CORES
    z_out = nc.dram_tensor("z_out", [B, dkr, S], F32,
                           kind="ExternalOutput").ap()

    with tile.TileContext(nc) as tc, ExitStack() as ctx:
        singles = ctx.enter_context(tc.tile_pool(name="singles", bufs=1))
        xt_pool = ctx.enter_context(tc.tile_pool(name="xt", bufs=3))
        qk_pool = ctx.enter_context(tc.tile_pool(name="qk", bufs=2))
        att_pool = ctx.enter_context(tc.tile_pool(name="attsb", bufs=3))
        pt_pool = ctx.enter_context(tc.tile_pool(name="pt", bufs=QB + 1))
        zc_pool = ctx.enter_context(tc.tile_pool(name="zc", bufs=2))
        small = ctx.enter_context(tc.tile_pool(name="small", bufs=2))
        psum = ctx.enter_context(tc.tile_pool(name="psum", bufs=6, space="PSUM"))
        psacc = ctx.enter_context(
            tc.tile_pool(name="psacc", bufs=2, space="PSUM"))
        dram = ctx.enter_context(tc.tile_pool(name="dram", bufs=8, space="DRAM"))
        dram_rec = ctx.enter_context(
            tc.tile_pool(name="dram_rec", bufs=4, space="DRAM"))

        # --- constants / weights ---
        wq_sb = singles.tile([128, DC, HF], F32R, tag="wq")
        wk_sb = singles.tile([128, DC, HF], F32R, tag="wk")
        wv_sb = singles.tile([128, DC, HF], F32R, tag="wv")
        nc.sync.dma_start(wq_sb[:], wq.rearrange("(c p) n -> p c n", p=128))
        nc.sync.dma_start(wk_sb[:], wk.rearrange("(c p) n -> p c n", p=128))
        nc.sync.dma_start(wv_sb[:], wv.rearrange("(c p) n -> p c n", p=128))
        bq_sb = singles.tile([HF, 1], F32, tag="bq")
        bk_sb = singles.tile([HF, 1], F32, tag="bk")
        bv_sb = singles.tile([HF, 1], F32, tag="bv")
        nc.sync.dma_start(bq_sb[:], bq[:])
        nc.sync.dma_start(bk_sb[:], bk[:])
        nc.sync.dma_start(bv_sb[:], bv[:])
        wo_sb = singles.tile([HF, dk], F32R, tag="wo")
        nc.sync.dma_start(wo_sb[:], wo[:])
        bo_sb = singles.tile([dkr, 1], F32, tag="bo")
        nc.sync.dma_start(bo_sb[:], bo[:])
        id_sb = singles.tile([128, 128], F32R, tag="ident")
        nc.sync.dma_start(id_sb[:], ident[:])

        ones_sb = singles.tile([128, hpc * QB], F32R, tag="ones")
        nc.vector.tensor_scalar(
            ones_sb[:], id_sb[:, 0:hpc * QB], scalar1=0.0, scalar2=1.0,
            op0=mybir.AluOpType.mult, op1=mybir.AluOpType.add)
        Vns = [singles.tile([128, hpc, QB, dk + 1], F32R, tag=f"Vn{i}",
                            name=f"Vn{i}")
               for i in range(2)]
        for i in range(2):
            nc.vector.tensor_copy(
                Vns[i][:, :, :, dk],
                ones_sb[:].rearrange("p (h q) -> p h q", h=hpc))

        rs_ins = [dram.tile([dk, S], F32, tag="rs_in", name=f"rsin{b}")
                  for b in range(B)]
        rs_outs = [dram.tile([dkr, S], F32, tag="rs_out", name=f"rsout{b}")
                   for b in range(B)]

        for b in range(B):
            ms = b * S

            # --- QKV projections (both heads packed on the partition dim) ---
            QTb = qk_pool.tile([HF, S], F32R, tag="QTb")
            KTb = qk_pool.tile([HF, S], F32R, tag="KTb")
            VTb = qk_pool.tile([HF, S], F32R, tag="VTb")
            for mt in range(NT):
                # x^T tile for this m-slice: [128, DC, TQ]
                xTm = xt_pool.tile([128, DC, TQ], F32R, tag="xTm")
                nc.sync.dma_start(xTm[:], xT[:, b, mt])
                for w_sb, b_sb, dst in ((wq_sb, bq_sb, QTb),
                                        (wk_sb, bk_sb, KTb),
                                        (wv_sb, bv_sb, VTb)):
                    ps = psum.tile([128, TQ], F32, tag="ps")
                    for c in range(DC):
                        nc.tensor.matmul(ps[:], w_sb[:, c, :], xTm[:, c, :],
                                         start=(c == 0), stop=(c == DC - 1))
                    nc.vector.tensor_scalar(
                        dst[:, mt * TQ:(mt + 1) * TQ], ps[:],
                        scalar1=b_sb[:], scalar2=0.0,
                        op0=mybir.AluOpType.add, op1=mybir.AluOpType.max)

            # --- V in natural [s, dk] layout (ones column pre-filled) ---
            Vnb = Vns[b % 2]
            for h in range(hpc):
                for sc in range(QB):
                    pst = psum.tile([128, TQ], F32R, tag="ps")
                    nc.tensor.transpose(
                        pst[:, :dk],
                        VTb[h * dk:(h + 1) * dk, sc * 128:(sc + 1) * 128],
                        id_sb[h * dk:(h + 1) * dk, h * dk:(h + 1) * dk])
                    nc.vector.tensor_copy(Vnb[:, h, sc, 0:dk], pst[:, :dk])

            zcb = zc_pool.tile([HF, S], F32R, tag="zcb")

            for h in range(hpc):
                QTh = QTb[h * dk:(h + 1) * dk, :]
                KTh = KTb[h * dk:(h + 1) * dk, :]

                # att^T[s, q]: raw scores (DMA'd transposed; host views
                # them back) AND exp() -> P^T, both evicted from one PSUM pass
                PTs = []
                for sb in range(QB):
                    ptt = pt_pool.tile([128, S], F32R, tag="pt")
                    attsb = att_pool.tile([128, S], F32, tag="attsb")
                    for qt in range(NT):
                        ps = psum.tile([128, TQ], F32, tag="ps")
                        nc.tensor.matmul(ps[:], KTh[:, sb * 128:(sb + 1) * 128],
                                         QTh[:, qt * TQ:(qt + 1) * TQ],
                                         start=True, stop=True)
                        nc.scalar.activation(ptt[:, qt * TQ:(qt + 1) * TQ],
                                             ps[:], AF.Exp,
                                             scale=float(inv_sqrt_dk))
                        nc.vector.tensor_copy(
                            attsb[:, qt * TQ:(qt + 1) * TQ], ps[:])
                    nc.sync.dma_start(
                        att_out[b, h, sb * 128:(sb + 1) * 128, :], attsb[:])
                    PTs.append(ptt)

                # o^T = V_aug^T @ P^T  (row dk = softmax denominators)
                ops = [psacc.tile([128, TQ], F32, tag="psacc",
                                  name=f"psov{b}_{h}_{qt}")
                       for qt in range(NT)]
                for sc in range(QB):
                    for qt in range(NT):
                        nc.tensor.matmul(
                            ops[qt][0:dk + 1, :], Vnb[:, h, sc, :],
                            PTs[sc][:, qt * TQ:(qt + 1) * TQ],
                            start=(sc == 0), stop=(sc == QB - 1))
                rec = small.tile([1, S], F32, tag="rec")
                osb = small.tile([dk + 1, S], F32, tag="osb")
                for qt in range(NT):
                    nc.scalar.activation(rec[:, qt * TQ:(qt + 1) * TQ],
                                         ops[qt][dk:dk + 1, :], AF.Ln)
                    nc.vector.tensor_copy(osb[:, qt * TQ:(qt + 1) * TQ],
                                          ops[qt][0:dk + 1, :])
                nc.scalar.activation(rec[:], rec[:], AF.Exp, scale=-1.0)
                rec_d = dram_rec.tile([S], F32, tag="rec_d")
                nc.sync.dma_start(rec_d[:], rec[:])
                recB = small.tile([dk, S], F32, tag="recB")
                rd_ap = rec_d[:]
                rd_bcast = bass.AP(tensor=rd_ap.tensor, offset=rd_ap.offset,
                                   ap=[[0, dk]] + list(rd_ap.ap))
                nc.gpsimd.dma_start(recB[:], rd_bcast)
                for qt in range(NT):
                    nc.vector.tensor_mul(
                        zcb[h * dk:(h + 1) * dk, qt * TQ:(qt + 1) * TQ],
                        osb[0:dk, qt * TQ:(qt + 1) * TQ],
                        recB[:, qt * TQ:(qt + 1) * TQ])

            # row-sharded final linear: partial z^T = wo_c^T @ zc^T
            for qt in range(NT):
                ps = psum.tile([128, TQ], F32, tag="ps")
                nc.tensor.matmul(ps[0:dk, :], wo_sb[:],
                                 zcb[:, qt * TQ:(qt + 1) * TQ],
                                 start=True, stop=True)
                zpsb = small.tile([dk, TQ], F32, tag="zp")
                nc.vector.tensor_copy(zpsb[:], ps[0:dk, :])
                nc.sync.dma_start(rs_ins[b][:, qt * TQ:(qt + 1) * TQ], zpsb[:])

            nc.gpsimd.collective_compute(
                "ReduceScatter", mybir.AluOpType.add,
                replica_groups=[list(range(N_CORES))],
                ins=[rs_ins[b][:].opt()], outs=[rs_outs[b][:].opt()])

            # tail: bias + relu on this core's slice of the reduced sum
            t = small.tile([dkr, S], F32, tag="tail_in")
            nc.sync.dma_start(t[:], rs_outs[b][:])
            t2 = small.tile([dkr, S], F32, tag="tail_out")
            nc.scalar.activation(t2[:], t[:], AF.Relu, bias=bo_sb[:])
            nc.sync.dma_start(z_out[b, :, :], t2[:])

    nc.compile()
    return nc


def _in_maps(input_vector, Wq, bq, Wk, bk, Wv, bv, Wo, bo, hpc):
    B, S, D = input_vector.shape
    dk = Wq.shape[2]
    TQ0 = 512 if S % 512 == 0 else 256
    x2dT = np.ascontiguousarray(
        input_vector.reshape(B, S // TQ0, TQ0, D // 128, 128).transpose(
            4, 0, 1, 3, 2)).astype(np.float32)
    ident = np.eye(128, dtype=np.float32)
    maps = []
    for c in range(N_CORES):
        h0 = c * hpc

        def pack_w(W):
            return np.ascontiguousarray(
                np.concatenate([W[h0 + j] for j in range(hpc)], axis=1))

        def pack_b(bias):
            return np.ascontiguousarray(
                np.concatenate([bias[h0 + j] for j in range(hpc)])[:, None])

        maps.append({
            "xT": x2dT,
            "wq": pack_w(Wq), "wk": pack_w(Wk), "wv": pack_w(Wv),
            "bq": pack_b(bq), "bk": pack_b(bk), "bv": pack_b(bv),
            "wo": np.ascontiguousarray(Wo[c * hpc * dk:(c + 1) * hpc * dk, :]),
            "bo": np.ascontiguousarray(
                bo[c * (dk // N_CORES):(c + 1) * (dk // N_CORES)][:, None]),
            "ident": ident,
        })
    return maps


_PROGRAM_CACHE = {}


def kernel(input_vector, Wq, bq, Wk, bk, Wv, bv, Wo, bo):
    B, S, D = input_vector.shape
    H, _, dk = Wq.shape
    hpc = H // N_CORES
    key = (B, S, D, dk, hpc)
    if key not in _PROGRAM_CACHE:
        _PROGRAM_CACHE[key] = build_program(B, S, D, dk=dk, hpc=hpc)
    nc = _PROGRAM_CACHE[key]
    maps = _in_maps(input_vector, Wq, bq, Wk, bk, Wv, bv, Wo, bo, hpc)
    res = run_bass_kernel_spmd(nc, maps, list(range(N_CORES))).results
    # att_out is stored [b, h, s, q]; transpose view back to [b, h, q, s]
    att = np.concatenate([res[c]["att_out"] for c in range(N_CORES)],
                         axis=1).transpose(0, 1, 3, 2)
    zT = np.concatenate([res[c]["z_out"] for c in range(N_CORES)], axis=1)
    z = np.ascontiguousarray(zT.transpose(0, 2, 1))
    return z, att


# revision 22
# speedup vs baseline: 1.0757x; 1.0757x over previous
"""Multi-head attention (16 heads, head-parallel over 8 NeuronCores) in Bass/Tile.

Sharding: 2 heads per core. Each core computes its heads' QKV projections,
raw attention scores (an output), softmax, attention-weighted values, and a
row-sharded slice of the final linear; partial products are summed with an
on-device AllReduce. Host only packs per-core weights, transposes x once,
and concatenates per-core att outputs.

Layout notes (T suffix = feature-major, i.e. transposed vs torch layout):
 - xT        [D, M]        M = B*S; moving operand for projections
 - QT/KT/VT  [128, S]      per b; rows 0:64 head0, 64:128 head1
 - att[q,s]  via matmul(lhsT=QT block, rhs=KT)     -> raw scores output
 - attT[s,q] via matmul(lhsT=KT block, rhs=QT)     -> exp() -> PT
 - o^T       via matmul(lhsT=V_aug[s, 65], rhs=PT) ; col 64 of V_aug is 1.0
              so o^T row 64 = softmax denominator (unnormalized row sums)
 - all matmul operands are float32r (FP22 mantissa) for 4x PE throughput;
   accumulation stays fp32 in PSUM.
"""

import sys

sys.path.insert(0, "/opt/trn_rl_repo")

from contextlib import ExitStack

import numpy as np

import concourse.bass as bass  # noqa: F401  (AP types)
import concourse.tile as tile
from concourse import bacc, mybir
from concourse.bass_utils import run_bass_kernel_spmd

F32 = mybir.dt.float32
F32R = mybir.dt.float32r
AF = mybir.ActivationFunctionType

N_CORES = 8


def build_program(B, S, D, dk=64, hpc=2):
    M = B * S
    DC = D // 128                      # contraction chunks for projections
    TQ = 512 if S % 512 == 0 else 256  # moving-operand tile size
    NT = S // TQ                       # moving tiles per sequence
    QB = S // 128                      # 128-row blocks per sequence
    HF = hpc * dk                      # features per core (128)
    assert HF == 128 and D % 128 == 0 and S % 256 == 0
    inv_sqrt_dk = 1.0 / float(np.sqrt(dk))

    nc = bacc.Bacc("TRN2", target_bir_lowering=False, debug=False,
                   num_devices=N_CORES)

    TQ0 = 512 if S % 512 == 0 else 256
    xT = nc.dram_tensor("xT", [128, B, S // TQ0, D // 128, TQ0], F32R,
                        kind="ExternalInput").ap()
    wq = nc.dram_tensor("wq", [D, HF], F32R, kind="ExternalInput").ap()
    wk = nc.dram_tensor("wk", [D, HF], F32R, kind="ExternalInput").ap()
    wv = nc.dram_tensor("wv", [D, HF], F32R, kind="ExternalInput").ap()
    bq = nc.dram_tensor("bq", [HF, 1], F32, kind="ExternalInput").ap()
    bk = nc.dram_tensor("bk", [HF, 1], F32, kind="ExternalInput").ap()
    bv = nc.dram_tensor("bv", [HF, 1], F32, kind="ExternalInput").ap()
    wo = nc.dram_tensor("wo", [HF, dk], F32R, kind="ExternalInput").ap()
    bo = nc.dram_tensor("bo", [dk // N_CORES, 1], F32,
                        kind="ExternalInput").ap()
    ident = nc.dram_tensor("ident", [128, 128], F32R, kind="ExternalInput").ap()
    att_out = nc.dram_tensor("att_out", [B, hpc, S, S], F32,
                             kind="ExternalOutput").ap()
    dkr = dk // N_CORES
    z_out = nc.dram_tensor("z_out", [B, dkr, S], F32,
                           kind="ExternalOutput").ap()

    with tile.TileContext(nc) as tc, ExitStack() as ctx:
        singles = ctx.enter_context(tc.tile_pool(name="singles", bufs=1))
        xt_pool = ctx.enter_context(tc.tile_pool(name="xt", bufs=3))
        qk_pool = ctx.enter_context(tc.tile_pool(name="qk", bufs=2))
        att_pool = ctx.enter_context(tc.tile_pool(name="attsb", bufs=3))
        pt_pool = ctx.enter_context(tc.tile_pool(name="pt", bufs=QB + 1))
        zc_pool = ctx.enter_context(tc.tile_pool(name="zc", bufs=2))
        small = ctx.enter_context(tc.tile_pool(name="small", bufs=2))
        psum = ctx.enter_context(tc.tile_pool(name="psum", bufs=8, space="PSUM"))
        dram = ctx.enter_context(tc.tile_pool(name="dram", bufs=8, space="DRAM"))
        dram_rec = ctx.enter_context(
            tc.tile_pool(name="dram_rec", bufs=4, space="DRAM"))

        # --- constants / weights ---
        wq_sb = singles.tile([128, DC, HF], F32R, tag="wq")
        wk_sb = singles.tile([128, DC, HF], F32R, tag="wk")
        wv_sb = singles.tile([128, DC, HF], F32R, tag="wv")
        nc.sync.dma_start(wq_sb[:], wq.rearrange("(c p) n -> p c n", p=128))
        nc.sync.dma_start(wk_sb[:], wk.rearrange("(c p) n -> p c n", p=128))
        nc.sync.dma_start(wv_sb[:], wv.rearrange("(c p) n -> p c n", p=128))
        bq_sb = singles.tile([HF, 1], F32, tag="bq")
        bk_sb = singles.tile([HF, 1], F32, tag="bk")
        bv_sb = singles.tile([HF, 1], F32, tag="bv")
        nc.sync.dma_start(bq_sb[:], bq[:])
        nc.sync.dma_start(bk_sb[:], bk[:])
        nc.sync.dma_start(bv_sb[:], bv[:])
        wo_sb = singles.tile([HF, dk], F32R, tag="wo")
        nc.sync.dma_start(wo_sb[:], wo[:])
        bo_sb = singles.tile([dkr, 1], F32, tag="bo")
        nc.sync.dma_start(bo_sb[:], bo[:])
        id_sb = singles.tile([128, 128], F32R, tag="ident")
        nc.sync.dma_start(id_sb[:], ident[:])

        ones_sb = singles.tile([128, hpc * QB], F32R, tag="ones")
        nc.vector.tensor_scalar(
            ones_sb[:], id_sb[:, 0:hpc * QB], scalar1=0.0, scalar2=1.0,
            op0=mybir.AluOpType.mult, op1=mybir.AluOpType.add)
        Vns = [singles.tile([128, hpc, QB, dk + 1], F32R, tag=f"Vn{i}",
                            name=f"Vn{i}")
               for i in range(2)]
        for i in range(2):
            nc.vector.tensor_copy(
                Vns[i][:, :, :, dk],
                ones_sb[:].rearrange("p (h q) -> p h q", h=hpc))

        rs_ins = [dram.tile([dk, S], F32, tag="rs_in", name=f"rsin{b}")
                  for b in range(B)]
        rs_outs = [dram.tile([dkr, S], F32, tag="rs_out", name=f"rsout{b}")
                   for b in range(B)]

        for b in range(B):
            ms = b * S

            # --- QKV projections (both heads packed on the partition dim) ---
            QTb = qk_pool.tile([HF, S], F32R, tag="QTb")
            KTb = qk_pool.tile([HF, S], F32R, tag="KTb")
            VTb = qk_pool.tile([HF, S], F32R, tag="VTb")
            for mt in range(NT):
                # x^T tile for this m-slice: [128, DC, TQ]
                xTm = xt_pool.tile([128, DC, TQ], F32R, tag="xTm")
                nc.sync.dma_start(xTm[:], xT[:, b, mt])
                for w_sb, b_sb, dst in ((wq_sb, bq_sb, QTb),
                                        (wk_sb, bk_sb, KTb),
                                        (wv_sb, bv_sb, VTb)):
                    ps = psum.tile([128, TQ], F32, tag="ps")
                    for c in range(DC):
                        nc.tensor.matmul(ps[:], w_sb[:, c, :], xTm[:, c, :],
                                         start=(c == 0), stop=(c == DC - 1))
                    nc.vector.tensor_scalar(
                        dst[:, mt * TQ:(mt + 1) * TQ], ps[:],
                        scalar1=b_sb[:], scalar2=0.0,
                        op0=mybir.AluOpType.add, op1=mybir.AluOpType.max)

            # --- V in natural [s, dk] layout (ones column pre-filled) ---
            Vnb = Vns[b % 2]
            for h in range(hpc):
                for sc in range(QB):
                    pst = psum.tile([128, TQ], F32R, tag="ps")
                    nc.tensor.transpose(
                        pst[:, :dk],
                        VTb[h * dk:(h + 1) * dk, sc * 128:(sc + 1) * 128],
                        id_sb[h * dk:(h + 1) * dk, h * dk:(h + 1) * dk])
                    nc.vector.tensor_copy(Vnb[:, h, sc, 0:dk], pst[:, :dk])

            zcb = zc_pool.tile([HF, S], F32R, tag="zcb")

            for h in range(hpc):
                QTh = QTb[h * dk:(h + 1) * dk, :]
                KTh = KTb[h * dk:(h + 1) * dk, :]

                # att^T[s, q]: raw scores (DMA'd transposed; host views
                # them back) AND exp() -> P^T, both evicted from one PSUM pass
                PTs = []
                for sb in range(QB):
                    ptt = pt_pool.tile([128, S], F32R, tag="pt")
                    attsb = att_pool.tile([128, S], F32, tag="attsb")
                    for qt in range(NT):
                        ps = psum.tile([128, TQ], F32, tag="ps")
                        nc.tensor.matmul(ps[:], KTh[:, sb * 128:(sb + 1) * 128],
                                         QTh[:, qt * TQ:(qt + 1) * TQ],
                                         start=True, stop=True)
                        nc.scalar.activation(ptt[:, qt * TQ:(qt + 1) * TQ],
                                             ps[:], AF.Exp,
                                             scale=float(inv_sqrt_dk))
                        sl = attsb[:, qt * TQ:(qt + 1) * TQ]
                        unit = ((b * hpc + h) * QB + sb) * NT + qt
                        if unit % 8 == 7:
                            nc.scalar.copy(sl, ps[:])
                        else:
                            nc.vector.tensor_copy(sl, ps[:])
                    nc.sync.dma_start(
                        att_out[b, h, sb * 128:(sb + 1) * 128, :], attsb[:])
                    PTs.append(ptt)

                # o^T = V_aug^T @ P^T  (row dk = softmax denominators)
                ops = [psum.tile([128, TQ], F32, tag="ps", name=f"psov{b}_{h}_{qt}")
                       for qt in range(NT)]
                for sc in range(QB):
                    for qt in range(NT):
                        nc.tensor.matmul(
                            ops[qt][0:dk + 1, :], Vnb[:, h, sc, :],
                            PTs[sc][:, qt * TQ:(qt + 1) * TQ],
                            start=(sc == 0), stop=(sc == QB - 1))
                rec = small.tile([1, S], F32, tag="rec")
                for qt in range(NT):
                    nc.scalar.activation(rec[:, qt * TQ:(qt + 1) * TQ],
                                         ops[qt][dk:dk + 1, :], AF.Ln)
                rec2 = small.tile([1, S], F32, tag="rec2")
                nc.scalar.activation(rec2[:], rec[:], AF.Exp, scale=-1.0)
                rec_d = dram_rec.tile([S], F32, tag="rec_d")
                nc.sync.dma_start(rec_d[:], rec2[:])
                recB = small.tile([dk, S], F32, tag="recB")
                rd_ap = rec_d[:]
                rd_bcast = bass.AP(tensor=rd_ap.tensor, offset=rd_ap.offset,
                                   ap=[[0, dk]] + list(rd_ap.ap))
                nc.gpsimd.dma_start(recB[:], rd_bcast)
                for qt in range(NT):
                    nc.vector.tensor_mul(
                        zcb[h * dk:(h + 1) * dk, qt * TQ:(qt + 1) * TQ],
                        ops[qt][0:dk, :], recB[:, qt * TQ:(qt + 1) * TQ])

            # row-sharded final linear: partial z^T = wo_c^T @ zc^T
            for qt in range(NT):
                ps = psum.tile([128, TQ], F32, tag="ps")
                nc.tensor.matmul(ps[0:dk, :], wo_sb[:],
                                 zcb[:, qt * TQ:(qt + 1) * TQ],
                                 start=True, stop=True)
                zpsb = small.tile([dk, TQ], F32, tag="zp")
                nc.vector.tensor_copy(zpsb[:], ps[0:dk, :])
                nc.sync.dma_start(rs_ins[b][:, qt * TQ:(qt + 1) * TQ], zpsb[:])

            nc.gpsimd.collective_compute(
                "ReduceScatter", mybir.AluOpType.add,
                replica_groups=[list(range(N_CORES))],
                ins=[rs_ins[b][:].opt()], outs=[rs_outs[b][:].opt()])

            # tail: bias + relu on this core's slice of the reduced sum
            t = small.tile([dkr, S], F32, tag="tail_in")
            nc.sync.dma_start(t[:], rs_outs[b][:])
            t2 = small.tile([dkr, S], F32, tag="tail_out")
            nc.scalar.activation(t2[:], t[:], AF.Relu, bias=bo_sb[:])
            nc.sync.dma_start(z_out[b, :, :], t2[:])

    nc.compile()
    return nc


def _in_maps(input_vector, Wq, bq, Wk, bk, Wv, bv, Wo, bo, hpc):
    B, S, D = input_vector.shape
    dk = Wq.shape[2]
    TQ0 = 512 if S % 512 == 0 else 256
    x2dT = np.ascontiguousarray(
        input_vector.reshape(B, S // TQ0, TQ0, D // 128, 128).transpose(
            4, 0, 1, 3, 2)).astype(np.float32)
    ident = np.eye(128, dtype=np.float32)
    maps = []
    for c in range(N_CORES):
        h0 = c * hpc

        def pack_w(W):
            return np.ascontiguousarray(
                np.concatenate([W[h0 + j] for j in range(hpc)], axis=1))

        def pack_b(bias):
            return np.ascontiguousarray(
                np.concatenate([bias[h0 + j] for j in range(hpc)])[:, None])

        maps.append({
            "xT": x2dT,
            "wq": pack_w(Wq), "wk": pack_w(Wk), "wv": pack_w(Wv),
            "bq": pack_b(bq), "bk": pack_b(bk), "bv": pack_b(bv),
            "wo": np.ascontiguousarray(Wo[c * hpc * dk:(c + 1) * hpc * dk, :]),
            "bo": np.ascontiguousarray(
                bo[c * (dk // N_CORES):(c + 1) * (dk // N_CORES)][:, None]),
            "ident": ident,
        })
    return maps


_PROGRAM_CACHE = {}


def kernel(input_vector, Wq, bq, Wk, bk, Wv, bv, Wo, bo):
    B, S, D = input_vector.shape
    H, _, dk = Wq.shape
    hpc = H // N_CORES
    key = (B, S, D, dk, hpc)
    if key not in _PROGRAM_CACHE:
        _PROGRAM_CACHE[key] = build_program(B, S, D, dk=dk, hpc=hpc)
    nc = _PROGRAM_CACHE[key]
    maps = _in_maps(input_vector, Wq, bq, Wk, bk, Wv, bv, Wo, bo, hpc)
    res = run_bass_kernel_spmd(nc, maps, list(range(N_CORES))).results
    # att_out is stored [b, h, s, q]; transpose view back to [b, h, q, s]
    att = np.concatenate([res[c]["att_out"] for c in range(N_CORES)],
                         axis=1).transpose(0, 1, 3, 2)
    zT = np.concatenate([res[c]["z_out"] for c in range(N_CORES)], axis=1)
    z = np.ascontiguousarray(zT.transpose(0, 2, 1))
    return z, att


# revision 23
# speedup vs baseline: 1.1025x; 1.0249x over previous
"""Multi-head attention (16 heads, head-parallel over 8 NeuronCores) in Bass/Tile.

Sharding: 2 heads per core. Each core computes its heads' QKV projections,
raw attention scores (an output), softmax, attention-weighted values, and a
row-sharded slice of the final linear; partial products are summed with an
on-device AllReduce. Host only packs per-core weights, transposes x once,
and concatenates per-core att outputs.

Layout notes (T suffix = feature-major, i.e. transposed vs torch layout):
 - xT        [D, M]        M = B*S; moving operand for projections
 - QT/KT/VT  [128, S]      per b; rows 0:64 head0, 64:128 head1
 - att[q,s]  via matmul(lhsT=QT block, rhs=KT)     -> raw scores output
 - attT[s,q] via matmul(lhsT=KT block, rhs=QT)     -> exp() -> PT
 - o^T       via matmul(lhsT=V_aug[s, 65], rhs=PT) ; col 64 of V_aug is 1.0
              so o^T row 64 = softmax denominator (unnormalized row sums)
 - all matmul operands are float32r (FP22 mantissa) for 4x PE throughput;
   accumulation stays fp32 in PSUM.
"""

import sys

sys.path.insert(0, "/opt/trn_rl_repo")

from contextlib import ExitStack

import numpy as np

import concourse.bass as bass  # noqa: F401  (AP types)
import concourse.tile as tile
from concourse import bacc, mybir
from concourse.bass_utils import run_bass_kernel_spmd

F32 = mybir.dt.float32
F32R = mybir.dt.float32r
AF = mybir.ActivationFunctionType

N_CORES = 8


def build_program(B, S, D, dk=64, hpc=2):
    M = B * S
    DC = D // 128                      # contraction chunks for projections
    TQ = 512 if S % 512 == 0 else 256  # moving-operand tile size
    NT = S // TQ                       # moving tiles per sequence
    QB = S // 128                      # 128-row blocks per sequence
    HF = hpc * dk                      # features per core (128)
    assert HF == 128 and D % 128 == 0 and S % 256 == 0
    inv_sqrt_dk = 1.0 / float(np.sqrt(dk))

    nc = bacc.Bacc("TRN2", target_bir_lowering=False, debug=False,
                   num_devices=N_CORES)

    TQ0 = 512 if S % 512 == 0 else 256
    xT = nc.dram_tensor("xT", [128, B, S // TQ0, D // 128, TQ0], F32R,
                        kind="ExternalInput").ap()
    wq = nc.dram_tensor("wq", [D, HF], F32R, kind="ExternalInput").ap()
    wk = nc.dram_tensor("wk", [D, HF], F32R, kind="ExternalInput").ap()
    wv = nc.dram_tensor("wv", [D, HF], F32R, kind="ExternalInput").ap()
    bq = nc.dram_tensor("bq", [HF, 1], F32, kind="ExternalInput").ap()
    bk = nc.dram_tensor("bk", [HF, 1], F32, kind="ExternalInput").ap()
    bv = nc.dram_tensor("bv", [HF, 1], F32, kind="ExternalInput").ap()
    wo = nc.dram_tensor("wo", [HF, dk], F32R, kind="ExternalInput").ap()
    bo = nc.dram_tensor("bo", [dk // N_CORES, 1], F32,
                        kind="ExternalInput").ap()
    ident = nc.dram_tensor("ident", [128, 128], F32R, kind="ExternalInput").ap()
    att_out = nc.dram_tensor("att_out", [B, hpc, S, S], F32,
                             kind="ExternalOutput").ap()
    dkr = dk // N_CORES
    z_out = nc.dram_tensor("z_out", [B, dkr, S], F32,
                           kind="ExternalOutput").ap()

    with tile.TileContext(nc) as tc, ExitStack() as ctx:
        singles = ctx.enter_context(tc.tile_pool(name="singles", bufs=1))
        xt_pool = ctx.enter_context(tc.tile_pool(name="xt", bufs=3))
        qk_pool = ctx.enter_context(tc.tile_pool(name="qk", bufs=2))
        att_pool = ctx.enter_context(tc.tile_pool(name="attsb", bufs=4))
        pt_pool = ctx.enter_context(tc.tile_pool(name="pt", bufs=QB + 1))
        zc_pool = ctx.enter_context(tc.tile_pool(name="zc", bufs=2))
        small = ctx.enter_context(tc.tile_pool(name="small", bufs=2))
        psum = ctx.enter_context(tc.tile_pool(name="psum", bufs=8, space="PSUM"))
        dram = ctx.enter_context(tc.tile_pool(name="dram", bufs=8, space="DRAM"))
        dram_rec = ctx.enter_context(
            tc.tile_pool(name="dram_rec", bufs=4, space="DRAM"))

        # --- constants / weights ---
        wq_sb = singles.tile([128, DC, HF], F32R, tag="wq")
        wk_sb = singles.tile([128, DC, HF], F32R, tag="wk")
        wv_sb = singles.tile([128, DC, HF], F32R, tag="wv")
        nc.sync.dma_start(wq_sb[:], wq.rearrange("(c p) n -> p c n", p=128))
        nc.sync.dma_start(wk_sb[:], wk.rearrange("(c p) n -> p c n", p=128))
        nc.sync.dma_start(wv_sb[:], wv.rearrange("(c p) n -> p c n", p=128))
        bq_sb = singles.tile([HF, 1], F32, tag="bq")
        bk_sb = singles.tile([HF, 1], F32, tag="bk")
        bv_sb = singles.tile([HF, 1], F32, tag="bv")
        nc.sync.dma_start(bq_sb[:], bq[:])
        nc.sync.dma_start(bk_sb[:], bk[:])
        nc.sync.dma_start(bv_sb[:], bv[:])
        wo_sb = singles.tile([HF, dk], F32R, tag="wo")
        nc.sync.dma_start(wo_sb[:], wo[:])
        bo_sb = singles.tile([dkr, 1], F32, tag="bo")
        nc.sync.dma_start(bo_sb[:], bo[:])
        id_sb = singles.tile([128, 128], F32R, tag="ident")
        nc.sync.dma_start(id_sb[:], ident[:])

        ones_sb = singles.tile([128, hpc * QB], F32R, tag="ones")
        nc.vector.tensor_scalar(
            ones_sb[:], id_sb[:, 0:hpc * QB], scalar1=0.0, scalar2=1.0,
            op0=mybir.AluOpType.mult, op1=mybir.AluOpType.add)
        Vns = [singles.tile([128, hpc, QB, dk + 1], F32R, tag=f"Vn{i}",
                            name=f"Vn{i}")
               for i in range(2)]
        for i in range(2):
            nc.vector.tensor_copy(
                Vns[i][:, :, :, dk],
                ones_sb[:].rearrange("p (h q) -> p h q", h=hpc))

        rs_ins = [dram.tile([dk, S], F32, tag="rs_in", name=f"rsin{b}")
                  for b in range(B)]
        rs_outs = [dram.tile([dkr, S], F32, tag="rs_out", name=f"rsout{b}")
                   for b in range(B)]

        for b in range(B):
            ms = b * S

            # --- QKV projections (both heads packed on the partition dim) ---
            QTb = qk_pool.tile([HF, S], F32R, tag="QTb")
            KTb = qk_pool.tile([HF, S], F32R, tag="KTb")
            VTb = qk_pool.tile([HF, S], F32R, tag="VTb")
            for mt in range(NT):
                # x^T tile for this m-slice: [128, DC, TQ]
                xTm = xt_pool.tile([128, DC, TQ], F32R, tag="xTm")
                nc.sync.dma_start(xTm[:], xT[:, b, mt])
                for w_sb, b_sb, dst in ((wq_sb, bq_sb, QTb),
                                        (wk_sb, bk_sb, KTb),
                                        (wv_sb, bv_sb, VTb)):
                    ps = psum.tile([128, TQ], F32, tag="ps")
                    for c in range(DC):
                        nc.tensor.matmul(ps[:], w_sb[:, c, :], xTm[:, c, :],
                                         start=(c == 0), stop=(c == DC - 1))
                    nc.vector.tensor_scalar(
                        dst[:, mt * TQ:(mt + 1) * TQ], ps[:],
                        scalar1=b_sb[:], scalar2=0.0,
                        op0=mybir.AluOpType.add, op1=mybir.AluOpType.max)

            # --- V in natural [s, dk] layout (ones column pre-filled) ---
            Vnb = Vns[b % 2]
            for h in range(hpc):
                for sc in range(QB):
                    pst = psum.tile([128, TQ], F32R, tag="ps")
                    nc.tensor.transpose(
                        pst[:, :dk],
                        VTb[h * dk:(h + 1) * dk, sc * 128:(sc + 1) * 128],
                        id_sb[h * dk:(h + 1) * dk, h * dk:(h + 1) * dk])
                    nc.vector.tensor_copy(Vnb[:, h, sc, 0:dk], pst[:, :dk])

            zcb = zc_pool.tile([HF, S], F32R, tag="zcb")

            for h in range(hpc):
                QTh = QTb[h * dk:(h + 1) * dk, :]
                KTh = KTb[h * dk:(h + 1) * dk, :]

                # att^T[s, q]: raw scores (DMA'd transposed; host views
                # them back) AND exp() -> P^T, both evicted from one PSUM pass
                PTs = []
                for sb in range(QB):
                    ptt = pt_pool.tile([128, S], F32R, tag="pt")
                    attsb = att_pool.tile([128, S], F32, tag="attsb")
                    for qt in range(NT):
                        ps = psum.tile([128, TQ], F32, tag="ps")
                        nc.tensor.matmul(ps[:], KTh[:, sb * 128:(sb + 1) * 128],
                                         QTh[:, qt * TQ:(qt + 1) * TQ],
                                         start=True, stop=True)
                        nc.scalar.activation(ptt[:, qt * TQ:(qt + 1) * TQ],
                                             ps[:], AF.Exp,
                                             scale=float(inv_sqrt_dk))
                        sl = attsb[:, qt * TQ:(qt + 1) * TQ]
                        unit = ((b * hpc + h) * QB + sb) * NT + qt
                        if unit % 8 == 7:
                            nc.scalar.copy(sl, ps[:])
                        else:
                            nc.vector.tensor_copy(sl, ps[:])
                    nc.sync.dma_start(
                        att_out[b, h, sb * 128:(sb + 1) * 128, :], attsb[:])
                    PTs.append(ptt)

                # o^T = V_aug^T @ P^T  (row dk = softmax denominators)
                ops = [psum.tile([128, TQ], F32, tag="ps", name=f"psov{b}_{h}_{qt}")
                       for qt in range(NT)]
                for sc in range(QB):
                    for qt in range(NT):
                        nc.tensor.matmul(
                            ops[qt][0:dk + 1, :], Vnb[:, h, sc, :],
                            PTs[sc][:, qt * TQ:(qt + 1) * TQ],
                            start=(sc == 0), stop=(sc == QB - 1))
                rec = small.tile([1, S], F32, tag="rec")
                for qt in range(NT):
                    nc.scalar.activation(rec[:, qt * TQ:(qt + 1) * TQ],
                                         ops[qt][dk:dk + 1, :], AF.Ln)
                nc.scalar.activation(rec[:], rec[:], AF.Exp, scale=-1.0)
                rec_d = dram_rec.tile([S], F32, tag="rec_d")
                nc.sync.dma_start(rec_d[:], rec[:])
                recB = small.tile([dk, S], F32, tag="recB")
                rd_ap = rec_d[:]
                rd_bcast = bass.AP(tensor=rd_ap.tensor, offset=rd_ap.offset,
                                   ap=[[0, dk]] + list(rd_ap.ap))
                nc.gpsimd.dma_start(recB[:], rd_bcast)
                for qt in range(NT):
                    nc.vector.tensor_mul(
                        zcb[h * dk:(h + 1) * dk, qt * TQ:(qt + 1) * TQ],
                        ops[qt][0:dk, :], recB[:, qt * TQ:(qt + 1) * TQ])

            # row-sharded final linear: partial z^T = wo_c^T @ zc^T
            for qt in range(NT):
                ps = psum.tile([128, TQ], F32, tag="ps")
                nc.tensor.matmul(ps[0:dk, :], wo_sb[:],
                                 zcb[:, qt * TQ:(qt + 1) * TQ],
                                 start=True, stop=True)
                zpsb = small.tile([dk, TQ], F32, tag="zp")
                nc.vector.tensor_copy(zpsb[:], ps[0:dk, :])
                nc.sync.dma_start(rs_ins[b][:, qt * TQ:(qt + 1) * TQ], zpsb[:])

            nc.gpsimd.collective_compute(
                "ReduceScatter", mybir.AluOpType.add,
                replica_groups=[list(range(N_CORES))],
                ins=[rs_ins[b][:].opt()], outs=[rs_outs[b][:].opt()])

            # tail: bias + relu on this core's slice of the reduced sum
            t = small.tile([dkr, S], F32, tag="tail_in")
            nc.sync.dma_start(t[:], rs_outs[b][:])
            t2 = small.tile([dkr, S], F32, tag="tail_out")
            nc.scalar.activation(t2[:], t[:], AF.Relu, bias=bo_sb[:])
            nc.sync.dma_start(z_out[b, :, :], t2[:])

    nc.compile()
    return nc


def _in_maps(input_vector, Wq, bq, Wk, bk, Wv, bv, Wo, bo, hpc):
    B, S, D = input_vector.shape
    dk = Wq.shape[2]
    TQ0 = 512 if S % 512 == 0 else 256
    x2dT = np.ascontiguousarray(
        input_vector.reshape(B, S // TQ0, TQ0, D // 128, 128).transpose(
            4, 0, 1, 3, 2)).astype(np.float32)
    ident = np.eye(128, dtype=np.float32)
    maps = []
    for c in range(N_CORES):
        h0 = c * hpc

        def pack_w(W):
            return np.ascontiguousarray(
                np.concatenate([W[h0 + j] for j in range(hpc)], axis=1))

        def pack_b(bias):
            return np.ascontiguousarray(
                np.concatenate([bias[h0 + j] for j in range(hpc)])[:, None])

        maps.append({
            "xT": x2dT,
            "wq": pack_w(Wq), "wk": pack_w(Wk), "wv": pack_w(Wv),
            "bq": pack_b(bq), "bk": pack_b(bk), "bv": pack_b(bv),
            "wo": np.ascontiguousarray(Wo[c * hpc * dk:(c + 1) * hpc * dk, :]),
            "bo": np.ascontiguousarray(
                bo[c * (dk // N_CORES):(c + 1) * (dk // N_CORES)][:, None]),
            "ident": ident,
        })
    return maps


_PROGRAM_CACHE = {}


def kernel(input_vector, Wq, bq, Wk, bk, Wv, bv, Wo, bo):
    B, S, D = input_vector.shape
    H, _, dk = Wq.shape
    hpc = H // N_CORES
    key = (B, S, D, dk, hpc)
    if key not in _PROGRAM_CACHE:
        _PROGRAM_CACHE[key] = build_program(B, S, D, dk=dk, hpc=hpc)
    nc = _PROGRAM_CACHE[key]
    maps = _in_maps(input_vector, Wq, bq, Wk, bk, Wv, bv, Wo, bo, hpc)
    res = run_bass_kernel_spmd(nc, maps, list(range(N_CORES))).results
    # att_out is stored [b, h, s, q]; transpose view back to [b, h, q, s]
    att = np.concatenate([res[c]["att_out"] for c in range(N_CORES)],
                         axis=1).transpose(0, 1, 3, 2)
    zT = np.concatenate([res[c]["z_out"] for c in range(N_CORES)], axis=1)
    z = np.ascontiguousarray(zT.transpose(0, 2, 1))
    return z, att
